# revision 1
# baseline (speedup 1.0000x reference)
"""ArcFace logits on 8 Trainium2 NeuronCores — class-parallel (partial-FC) sharding.

Math: logits = SCALE * cos(arccos(clip(f_n @ w_n.T)) + MARGIN*onehot(targets))
Since cos(arccos(x)) == x, only the 1024 target entries need the margin
correction cos(t+m) = cos(m)*x - sin(m)*sqrt(1-x^2); everything else is just
the normalized matmul scaled by SCALE.

Device (SPMD, identical graph on all 8 cores, class-sharded):
  - w column norms via ACT Square + ones-matmul (PE) + DVE reciprocal + ACT Sqrt
  - f row-normalize (*SCALE folded in), cast bf16, PE-transpose -> fT
  - main matmul out[c,b] = wT.T @ fT in bf16 (fp32 PSUM), w-norm scaling fused
    into the PSUM->SBUF evacuation (per-partition tensor_scalar)
  - margin deltas for all 1024 rows from gathered target weight rows
Host: shard/transpose/concat + apply the device-computed deltas at the 1024
target positions (pure indexing; all arithmetic happens on device).
"""

import math
import os

import numpy as np

IN_F = 512
OUT_C = 100000
B = 1024
MARGIN = 0.5
SCALE = 20.0

NCORES = 8
CSH = 12800            # classes per core after padding
CPAD = CSH * NCORES    # 102400
P = 128
KT = IN_F // P         # 4 contraction subtiles
BT = B // P            # 8 batch tiles
NF = 512               # matmul moving free dim (one PSUM bank of fp32)
NB = B // NF           # 2
CW = 1280              # class chunk width streamed from DRAM
CBK = CW // P          # 10 class blocks per chunk
CHUNKS = CSH // CW     # 10

_GRAPH = None
LAST_EXEC_TIME_NS = None


def _build_graph():
    from contextlib import ExitStack

    import concourse.bass as bass  # noqa: F401
    import concourse.tile as tile
    from concourse import bacc, mybir
    from concourse.masks import make_identity

    dt = mybir.dt
    AF = mybir.ActivationFunctionType
    ALU = mybir.AluOpType
    cosm = math.cos(MARGIN)
    sinm = math.sin(MARGIN)

    nc = bacc.Bacc()
    wT_e = nc.declare_dram_parameter("wT", [IN_F, CSH], dt.bfloat16, isOutput=False)
    f_e = nc.declare_dram_parameter("f", [B, IN_F], dt.float32, isOutput=False)
    wtg_e = nc.declare_dram_parameter("wtgt", [B, IN_F], dt.float32, isOutput=False)
    out_e = nc.declare_dram_parameter("out", [CSH, B], dt.float32, isOutput=True)
    dlt_e = nc.declare_dram_parameter("delta", [P, BT], dt.float32, isOutput=True)

    wT_v = wT_e[:].rearrange("(k p) c -> p k c", p=P)   # d = k*128 + p
    f_v = f_e[:].rearrange("(t p) d -> p t d", p=P)     # b = t*128 + p
    wtg_v = wtg_e[:].rearrange("(t p) d -> p t d", p=P)

    with ExitStack() as ctx:
        tc = ctx.enter_context(tile.TileContext(nc))
        cpool = ctx.enter_context(tc.tile_pool(name="cpool", bufs=1))
        fpool = ctx.enter_context(tc.tile_pool(name="fpool", bufs=1))
        wpool = ctx.enter_context(tc.tile_pool(name="wpool", bufs=3))
        sqpool = ctx.enter_context(tc.tile_pool(name="sqpool", bufs=2))
        opool = ctx.enter_context(tc.tile_pool(name="opool", bufs=2))
        smal = ctx.enter_context(tc.tile_pool(name="smal", bufs=2))
        pt_pool = ctx.enter_context(tc.tile_pool(name="pt", bufs=2, space="PSUM"))
        pn_pool = ctx.enter_context(tc.tile_pool(name="pn", bufs=1, space="PSUM"))
        po_pool = ctx.enter_context(tc.tile_pool(name="po", bufs=5, space="PSUM"))

        ident = cpool.tile([P, P], dt.bfloat16)
        make_identity(nc, ident[:])
        ones_b = cpool.tile([P, 1], dt.bfloat16)
        nc.gpsimd.memset(ones_b[:], 1.0)

        # ---------------- f path: normalize, *SCALE, cast bf16, transpose ---
        f_sb = fpool.tile([P, BT, IN_F], dt.float32)
        nc.sync.dma_start(f_sb[:], f_v)

        nf2 = smal.tile([P, BT], dt.float32)
        for t in range(BT):
            sq = sqpool.tile([P, IN_F], dt.float32, tag="sqscratch")
            nc.scalar.activation(
                sq[:], f_sb[:, t], AF.Square, accum_out=nf2[:, t : t + 1]
            )
        rec_f = smal.tile([P, BT], dt.float32)
        nc.vector.reciprocal(rec_f[:], nf2[:])
        rnf20 = smal.tile([P, BT], dt.float32)
        # sqrt(SCALE^2 / nf2) = SCALE * rsqrt(nf2)
        nc.scalar.activation(rnf20[:], rec_f[:], AF.Sqrt, scale=SCALE * SCALE)

        f_n = fpool.tile([P, BT, IN_F], dt.bfloat16)
        for t in range(BT):
            nc.vector.tensor_scalar_mul(f_n[:, t], f_sb[:, t], rnf20[:, t : t + 1])

        fT = fpool.tile([P, KT, B], dt.bfloat16)
        for t in range(BT):
            for k in range(KT):
                ps = pt_pool.tile([P, P], dt.bfloat16, tag="pst")
                nc.tensor.transpose(ps[:], f_n[:, t, k * P : (k + 1) * P], ident[:])
                nc.vector.tensor_copy(fT[:, k, t * P : (t + 1) * P], ps[:])

        # ---------------- main class loop (norm phase pipelined 1 chunk ahead) --
        OG = 5                      # c-blocks batched per output DMA
        assert CBK % OG == 0

        def emit_load_square(ci):
            w_sb = wpool.tile([P, KT, CW], dt.bfloat16, tag="wchunk", name="w_sb")
            # chunk 0 on the fast-start HWDGE ring (SWDGE first-issue pays Q7
            # setup and queues behind the constant memsets); later prefetches
            # on SWDGE, off the HWDGE rings that carry the 2.5MB output bursts
            eng = nc.sync if ci == 0 else nc.gpsimd
            eng.dma_start(w_sb[:], wT_v[:, :, ci * CW : (ci + 1) * CW])
            wsq = sqpool.tile([P, KT, CW], dt.bfloat16, tag="wsq", name="wsq")
            nc.scalar.activation(wsq[:], w_sb[:], AF.Square)
            return w_sb, wsq

        def emit_norm(wsq):
            # per-class norm^2 -> [128, CBK] (classes on partitions)
            nps = pn_pool.tile([P, CBK], dt.float32, tag="nps", name="nps")
            for cb in range(CBK):
                for k in range(KT):
                    nc.tensor.matmul(
                        nps[:, cb : cb + 1],
                        lhsT=wsq[:, k, cb * P : (cb + 1) * P],
                        rhs=ones_b[:],
                        start=(k == 0),
                        stop=(k == KT - 1),
                    )
            recw = smal.tile([P, CBK], dt.float32, tag="recw", name="recw")
            nc.vector.reciprocal(recw[:], nps[:])
            rnw = smal.tile([P, CBK], dt.float32, tag="rnw", name="rnw")
            nc.scalar.activation(rnw[:], recw[:], AF.Sqrt, scale=1.0)
            return rnw

        # (margin path emitted mid-loop via emit_margin below)
        # margin input DMA issued early on the (idle) scalar HWDGE ring so it
        # is not queued behind the main loop's output DMAs
        wt_sb = fpool.tile([P, BT, IN_F], dt.float32, name="wt_sb")
        nc.scalar.dma_start(wt_sb[:], wtg_v)

        def emit_margin():
            nt2 = smal.tile([P, BT], dt.float32)
            drot = smal.tile([P, BT], dt.float32)
            for t in range(BT):
                sq = sqpool.tile([P, IN_F], dt.float32, tag="sqscratch")
                nc.scalar.activation(
                    sq[:], wt_sb[:, t], AF.Square, accum_out=nt2[:, t : t + 1]
                )
                prod = sqpool.tile([P, IN_F], dt.float32, tag="prodscratch")
                nc.vector.tensor_mul(prod[:], f_sb[:, t], wt_sb[:, t])
                nc.vector.reduce_sum(
                    drot[:, t : t + 1], prod[:], axis=mybir.AxisListType.X
                )
            rec_t = smal.tile([P, BT], dt.float32)
            nc.vector.reciprocal(rec_t[:], nt2[:])
            rnt = smal.tile([P, BT], dt.float32)
            nc.scalar.activation(rnt[:], rec_t[:], AF.Sqrt, scale=1.0)
            u = smal.tile([P, BT], dt.float32)
            nc.vector.tensor_mul(u[:], drot[:], rnf20[:])
            nc.vector.tensor_mul(u[:], u[:], rnt[:])          # u = SCALE * cos_t
            t1 = smal.tile([P, BT], dt.float32)
            nc.vector.tensor_mul(t1[:], u[:], u[:])
            nc.vector.tensor_scalar(t1[:], t1[:], -1.0, SCALE * SCALE, ALU.mult, ALU.add)
            nc.vector.tensor_scalar_max(t1[:], t1[:], 0.0)    # max(S^2 - u^2, 0)
            s_t = smal.tile([P, BT], dt.float32)
            nc.scalar.activation(s_t[:], t1[:], AF.Sqrt, scale=1.0)  # SCALE*sin_t
            t2 = smal.tile([P, BT], dt.float32)
            nc.vector.tensor_scalar_mul(t2[:], s_t[:], -sinm)
            t3 = smal.tile([P, BT], dt.float32)
            nc.vector.tensor_scalar_mul(t3[:], u[:], cosm - 1.0)
            delta = smal.tile([P, BT], dt.float32)
            nc.vector.tensor_add(delta[:], t2[:], t3[:])
            nc.sync.dma_start(dlt_e[:], delta[:])

        state = {}
        w_sb0, wsq0 = emit_load_square(0)
        state[0] = (w_sb0, emit_norm(wsq0))

        for ci in range(CHUNKS):
            w_sb, rnw = state.pop(ci)
            next_sq = None
            for og in range(CBK // OG):
                if og == 0 and ci + 1 < CHUNKS:
                    # prefetch + square the next chunk before this chunk's
                    # evacuations occupy ACT
                    next_sq = emit_load_square(ci + 1)
                elif og == 1 and next_sq is not None:
                    # norm matmuls for the next chunk run on PE between this
                    # chunk's output groups; recip/sqrt land before they're
                    # needed by the next chunk's evacuations
                    state[ci + 1] = (next_sq[0], emit_norm(next_sq[1]))
                osb = opool.tile([P, OG, NB * NF], dt.float32, tag="osb")
                for cbi in range(OG):
                    cb = og * OG + cbi
                    psos = [
                        po_pool.tile([P, NF], dt.float32, tag="pso", name=f"pso{nb}")
                        for nb in range(NB)
                    ]
                    for k in range(KT):
                        for nb in range(NB):
                            nc.tensor.matmul(
                                psos[nb][:],
                                lhsT=w_sb[:, k, cb * P : (cb + 1) * P],
                                rhs=fT[:, k, nb * NF : (nb + 1) * NF],
                                start=(k == 0),
                                stop=(k == KT - 1),
                            )
                    for nb in range(NB):
                        if (cb + nb) % 2 == 0:
                            nc.scalar.activation(
                                osb[:, cbi, nb * NF : (nb + 1) * NF],
                                psos[nb][:], AF.Copy,
                                scale=rnw[:, cb : cb + 1],
                            )
                        else:
                            nc.vector.tensor_scalar_mul(
                                osb[:, cbi, nb * NF : (nb + 1) * NF],
                                psos[nb][:], rnw[:, cb : cb + 1],
                            )
                row0 = ci * CW + og * OG * P
                dma_eng = nc.sync if og % 2 == 0 else nc.scalar
                dma_eng.dma_start(
                    out_e[row0 : row0 + OG * P, :].rearrange(
                        "(g p) b -> p g b", p=P
                    ),
                    osb[:],
                )


        emit_margin()

    nc.finalize()
    return nc


def _prep_inputs(features, targets, weights):
    import ml_dtypes

    f32 = np.ascontiguousarray(np.asarray(features, dtype=np.float32))
    tgt = np.asarray(targets).astype(np.int64)
    w = np.asarray(weights, dtype=np.float32)

    wpad = np.zeros((CPAD, IN_F), dtype=np.float32)
    wpad[:OUT_C] = w
    wpad[OUT_C:, 0] = 1.0  # unit-norm filler rows: no inf/nan anywhere

    in_maps = []
    for i in range(NCORES):
        sh = wpad[i * CSH : (i + 1) * CSH]
        wT = np.ascontiguousarray(sh.astype(ml_dtypes.bfloat16).T)
        loc = np.clip(tgt - i * CSH, 0, CSH - 1)
        wtgt = np.ascontiguousarray(sh[loc])
        in_maps.append({"wT": wT, "f": f32, "wtgt": wtgt})
    return in_maps, tgt


_LDW_PATCHED = False


def _maybe_enable_ldw_opt():
    """Optionally re-enable walrus's LDWEIGHTS dedup pass (env BASS_LDW_OPT=1)."""
    global _LDW_PATCHED
    if _LDW_PATCHED or os.environ.get("BASS_LDW_OPT", "0") != "1":
        return
    import concourse.bass_utils as bu

    orig = bu.run_command

    def run_command_ldw(cmd, *a, **kw):
        cmd = [
            c.replace("--enable-ldw-opt=false", "--enable-ldw-opt=true")
            if isinstance(c, str)
            else c
            for c in cmd
        ]
        return orig(cmd, *a, **kw)

    bu.run_command = run_command_ldw
    _LDW_PATCHED = True


def kernel(features, targets, weights):
    global _GRAPH, LAST_EXEC_TIME_NS
    from concourse.bass_utils import run_bass_kernel_spmd

    _maybe_enable_ldw_opt()

    if _GRAPH is None:
        _GRAPH = _build_graph()
    nc = _GRAPH

    in_maps, tgt = _prep_inputs(features, targets, weights)

    trace = bool(int(os.environ.get("BASS_KERNEL_TRACE", "0")))
    res = run_bass_kernel_spmd(nc, in_maps, core_ids=list(range(NCORES)), trace=trace)
    LAST_EXEC_TIME_NS = res.exec_time_ns

    outs = [res.results[i]["out"] for i in range(NCORES)]       # [CSH, B] each
    full = np.concatenate(outs, axis=0)[:OUT_C]                 # [OUT_C, B]
    logits = np.ascontiguousarray(full.T, dtype=np.float32)     # [B, OUT_C]

    # apply device-computed margin deltas at the 1024 target positions
    deltas = np.stack(
        [res.results[i]["delta"].T.reshape(B) for i in range(NCORES)]
    )  # [NCORES, B]; delta[p, t] -> b = t*128 + p
    rows = np.arange(B)
    core_of = (tgt // CSH).astype(np.int64)
    logits[rows, tgt] += deltas[core_of, rows]
    return logits



# revision 2
# speedup vs baseline: 1.0445x; 1.0445x over previous
"""ArcFace logits on 8 Trainium2 NeuronCores — class-parallel (partial-FC) sharding.

Math: logits = SCALE * cos(arccos(clip(f_n @ w_n.T)) + MARGIN*onehot(targets))
Since cos(arccos(x)) == x, only the 1024 target entries need the margin
correction cos(t+m) = cos(m)*x - sin(m)*sqrt(1-x^2); everything else is just
the normalized matmul scaled by SCALE.

Device (SPMD, identical graph on all 8 cores, class-sharded):
  - f row-normalize (*SCALE folded in), cast bf16, PE-transpose -> fT
  - main matmul out[c,b] = wT.T @ fT in bf16 (fp32 PSUM), w-norm scaling fused
    into the PSUM->SBUF evacuation (per-partition tensor_scalar), fp16 out
  - w column norms: ACT Square -> GPSIMD k-presum -> one N=1 ones-matmul per
    128-class block -> DVE reciprocal + ACT Sqrt
  - margin deltas for all 1024 rows from gathered target weight rows
DMA: weights + outputs stream on the two HWDGE rings (sync/scalar), weight
chunks prefetched two ahead; margin inputs ride the software DGE (gpsimd).
Host: shard/transpose/concat + apply the device-computed deltas at the 1024
target positions (pure indexing; all arithmetic happens on device).
"""

import math
import os

import numpy as np

IN_F = 512
OUT_C = 100000
B = 1024
MARGIN = 0.5
SCALE = 20.0

NCORES = 8
CSH = 12800            # classes per core after padding
CPAD = CSH * NCORES    # 102400
P = 128
KT = IN_F // P         # 4 contraction subtiles
BT = B // P            # 8 batch tiles
NF = 512               # matmul moving free dim (one PSUM bank of fp32)
NB = B // NF           # 2
CW = 1280              # class chunk width streamed from DRAM
CBK = CW // P          # 10 class blocks per chunk
CHUNKS = CSH // CW     # 10
OG = 5                 # c-blocks batched per output DMA

_GRAPH = None
LAST_EXEC_TIME_NS = None


def _build_graph():
    from contextlib import ExitStack

    import concourse.bass as bass  # noqa: F401
    import concourse.tile as tile
    from concourse import bacc, mybir
    from concourse.masks import make_identity

    dt = mybir.dt
    AF = mybir.ActivationFunctionType
    ALU = mybir.AluOpType
    cosm = math.cos(MARGIN)
    sinm = math.sin(MARGIN)

    nc = bacc.Bacc()
    wT_e = nc.declare_dram_parameter("wT", [IN_F, CSH], dt.bfloat16, isOutput=False)
    f_e = nc.declare_dram_parameter("f", [B, IN_F], dt.float32, isOutput=False)
    wtg_e = nc.declare_dram_parameter("wtgt", [B, IN_F], dt.float32, isOutput=False)
    out_e = nc.declare_dram_parameter("out", [CSH, B], dt.float16, isOutput=True)
    dlt_e = nc.declare_dram_parameter("delta", [P, BT], dt.float32, isOutput=True)

    wT_v = wT_e[:].rearrange("(k p) c -> p k c", p=P)   # d = k*128 + p
    f_v = f_e[:].rearrange("(t p) d -> p t d", p=P)     # b = t*128 + p
    wtg_v = wtg_e[:].rearrange("(t p) d -> p t d", p=P)

    with ExitStack() as ctx:
        tc = ctx.enter_context(tile.TileContext(nc))
        cpool = ctx.enter_context(tc.tile_pool(name="cpool", bufs=1))
        fpool = ctx.enter_context(tc.tile_pool(name="fpool", bufs=1))
        wpool = ctx.enter_context(tc.tile_pool(name="wpool", bufs=3))
        sqpool = ctx.enter_context(tc.tile_pool(name="sqpool", bufs=2))
        opool = ctx.enter_context(tc.tile_pool(name="opool", bufs=3))
        smal = ctx.enter_context(tc.tile_pool(name="smal", bufs=2))
        pt_pool = ctx.enter_context(tc.tile_pool(name="pt", bufs=2, space="PSUM"))
        pn_pool = ctx.enter_context(tc.tile_pool(name="pn", bufs=1, space="PSUM"))
        po_pool = ctx.enter_context(tc.tile_pool(name="po", bufs=5, space="PSUM"))

        # ---------------- input DMAs first: f split across both HWDGE rings,
        # w chunk0/chunk1 behind them, margin weight rows on the SWDGE ring ---
        f_sb = fpool.tile([P, BT, IN_F], dt.float32)
        nc.sync.dma_start(f_sb[:, 0 : BT // 2], f_v[:, 0 : BT // 2])
        nc.scalar.dma_start(f_sb[:, BT // 2 :], f_v[:, BT // 2 :])

        def emit_load(ci):
            w_sb = wpool.tile([P, KT, CW], dt.bfloat16, tag="wchunk", name="w_sb")
            eng = nc.sync if ci % 2 == 0 else nc.scalar
            eng.dma_start(w_sb[:], wT_v[:, :, ci * CW : (ci + 1) * CW])
            return w_sb

        w_sbs = {0: emit_load(0), 1: emit_load(1)}

        wt_sb = fpool.tile([P, BT, IN_F], dt.float32, name="wt_sb")
        nc.gpsimd.dma_start(wt_sb[:], wtg_v)

        ident = cpool.tile([P, P], dt.bfloat16)
        make_identity(nc, ident[:])
        ones_f = cpool.tile([P, 1], dt.float32)
        nc.gpsimd.memset(ones_f[:], 1.0)

        # ---------------- f path: normalize, *SCALE, cast bf16, transpose ---
        nf2 = smal.tile([P, BT], dt.float32)
        for t in range(BT):
            sq = sqpool.tile([P, IN_F], dt.float32, tag="sqscratch")
            nc.scalar.activation(
                sq[:], f_sb[:, t], AF.Square, accum_out=nf2[:, t : t + 1]
            )
        rec_f = smal.tile([P, BT], dt.float32)
        nc.vector.reciprocal(rec_f[:], nf2[:])
        rnf20 = smal.tile([P, BT], dt.float32)
        # sqrt(SCALE^2 / nf2) = SCALE * rsqrt(nf2)
        nc.scalar.activation(rnf20[:], rec_f[:], AF.Sqrt, scale=SCALE * SCALE)

        f_n = fpool.tile([P, BT, IN_F], dt.bfloat16)
        for t in range(BT):
            nc.vector.tensor_scalar_mul(f_n[:, t], f_sb[:, t], rnf20[:, t : t + 1])

        fT = fpool.tile([P, KT, B], dt.bfloat16)
        for t in range(BT):
            for k in range(KT):
                ps = pt_pool.tile([P, P], dt.bfloat16, tag="pst")
                nc.tensor.transpose(ps[:], f_n[:, t, k * P : (k + 1) * P], ident[:])
                nc.vector.tensor_copy(fT[:, k, t * P : (t + 1) * P], ps[:])

        # ---------------- w-norm pipeline pieces --------------------------
        def emit_sqsum(ci):
            """ACT square of chunk ci, then k-presum on GPSIMD -> wsum fp32."""
            w_sb = w_sbs[ci]
            wsq = sqpool.tile([P, KT, CW], dt.bfloat16, tag="wsq", name="wsq")
            nc.scalar.activation(wsq[:], w_sb[:], AF.Square)
            wsum = sqpool.tile([P, CW], dt.float32, tag="wsum", name="wsum")
            nc.gpsimd.tensor_add(wsum[:], wsq[:, 0], wsq[:, 1])
            nc.gpsimd.tensor_add(wsum[:], wsum[:], wsq[:, 2])
            nc.gpsimd.tensor_add(wsum[:], wsum[:], wsq[:, 3])
            return wsum

        def emit_normmm(ci, wsum):
            """Per-class norm^2 via one N=1 ones-matmul per 128-class block."""
            nps = pn_pool.tile([P, CBK], dt.float32, tag="nps", name="nps")
            for cb in range(CBK):
                nc.tensor.matmul(
                    nps[:, cb : cb + 1],
                    lhsT=wsum[:, cb * P : (cb + 1) * P],
                    rhs=ones_f[:],
                    start=True,
                    stop=True,
                )
            recw = smal.tile([P, CBK], dt.float32, tag="recw", name="recw")
            nc.vector.reciprocal(recw[:], nps[:])
            rnw = smal.tile([P, CBK], dt.float32, tag="rnw", name="rnw")
            nc.scalar.activation(rnw[:], recw[:], AF.Sqrt, scale=1.0)
            return rnw

        def emit_margin():
            nt2 = smal.tile([P, BT], dt.float32)
            drot = smal.tile([P, BT], dt.float32)
            for t in range(BT):
                sq = sqpool.tile([P, IN_F], dt.float32, tag="sqscratch")
                nc.scalar.activation(
                    sq[:], wt_sb[:, t], AF.Square, accum_out=nt2[:, t : t + 1]
                )
                prod = sqpool.tile([P, IN_F], dt.float32, tag="prodscratch")
                nc.vector.tensor_mul(prod[:], f_sb[:, t], wt_sb[:, t])
                nc.vector.reduce_sum(
                    drot[:, t : t + 1], prod[:], axis=mybir.AxisListType.X
                )
            rec_t = smal.tile([P, BT], dt.float32)
            nc.vector.reciprocal(rec_t[:], nt2[:])
            rnt = smal.tile([P, BT], dt.float32)
            nc.scalar.activation(rnt[:], rec_t[:], AF.Sqrt, scale=1.0)
            u = smal.tile([P, BT], dt.float32)
            nc.vector.tensor_mul(u[:], drot[:], rnf20[:])
            nc.vector.tensor_mul(u[:], u[:], rnt[:])          # u = SCALE * cos_t
            t1 = smal.tile([P, BT], dt.float32)
            nc.vector.tensor_mul(t1[:], u[:], u[:])
            nc.vector.tensor_scalar(t1[:], t1[:], -1.0, SCALE * SCALE, ALU.mult, ALU.add)
            nc.vector.tensor_scalar_max(t1[:], t1[:], 0.0)    # max(S^2 - u^2, 0)
            s_t = smal.tile([P, BT], dt.float32)
            nc.scalar.activation(s_t[:], t1[:], AF.Sqrt, scale=1.0)  # SCALE*sin_t
            t2 = smal.tile([P, BT], dt.float32)
            nc.vector.tensor_scalar_mul(t2[:], s_t[:], -sinm)
            t3 = smal.tile([P, BT], dt.float32)
            nc.vector.tensor_scalar_mul(t3[:], u[:], cosm - 1.0)
            delta = smal.tile([P, BT], dt.float32)
            nc.vector.tensor_add(delta[:], t2[:], t3[:])
            nc.gpsimd.dma_start(dlt_e[:], delta[:])

        # ---------------- main class loop ---------------------------------
        # chunk0's norm runs in the preamble (PE waits on it once);
        # chunk i+1's square/presum is emitted at og1 of chunk i and its
        # norm-matmuls at the end of chunk i, so steady-state PE never waits.
        rnw = emit_normmm(0, emit_sqsum(0))

        for ci in range(CHUNKS):
            w_sb = w_sbs.pop(ci)
            for og in range(CBK // OG):
                if og == 0 and ci + 2 < CHUNKS:
                    w_sbs[ci + 2] = emit_load(ci + 2)
                elif og == 1 and ci + 1 < CHUNKS:
                    next_wsum = emit_sqsum(ci + 1)
                osb = opool.tile([P, OG, NB * NF], dt.float16, tag="osb")
                for cbi in range(OG):
                    cb = og * OG + cbi
                    psos = [
                        po_pool.tile([P, NF], dt.float32, tag="pso", name=f"pso{nb}")
                        for nb in range(NB)
                    ]
                    for k in range(KT):
                        for nb in range(NB):
                            nc.tensor.matmul(
                                psos[nb][:],
                                lhsT=w_sb[:, k, cb * P : (cb + 1) * P],
                                rhs=fT[:, k, nb * NF : (nb + 1) * NF],
                                start=(k == 0),
                                stop=(k == KT - 1),
                            )
                    for nb in range(NB):
                        if (cb + nb) % 2 == 0:
                            nc.scalar.activation(
                                osb[:, cbi, nb * NF : (nb + 1) * NF],
                                psos[nb][:], AF.Copy,
                                scale=rnw[:, cb : cb + 1],
                            )
                        else:
                            nc.vector.tensor_scalar_mul(
                                osb[:, cbi, nb * NF : (nb + 1) * NF],
                                psos[nb][:], rnw[:, cb : cb + 1],
                            )
                row0 = ci * CW + og * OG * P
                if ci == CHUNKS - 1 and og == 1:
                    # split the final burst across both rings to cut the tail
                    nc.sync.dma_start(
                        out_e[row0 : row0 + 2 * P, :].rearrange(
                            "(g p) b -> p g b", p=P
                        ),
                        osb[:, 0:2],
                    )
                    nc.scalar.dma_start(
                        out_e[row0 + 2 * P : row0 + OG * P, :].rearrange(
                            "(g p) b -> p g b", p=P
                        ),
                        osb[:, 2:OG],
                    )
                else:
                    dma_eng = nc.sync if og % 2 == 0 else nc.scalar
                    dma_eng.dma_start(
                        out_e[row0 : row0 + OG * P, :].rearrange(
                            "(g p) b -> p g b", p=P
                        ),
                        osb[:],
                    )
            if ci + 1 < CHUNKS:
                rnw = emit_normmm(ci + 1, next_wsum)
            if ci == 3:
                emit_margin()

    nc.finalize()
    return nc


def _prep_inputs(features, targets, weights):
    import ml_dtypes

    f32 = np.ascontiguousarray(np.asarray(features, dtype=np.float32))
    tgt = np.asarray(targets).astype(np.int64)
    w = np.asarray(weights, dtype=np.float32)

    wpad = np.zeros((CPAD, IN_F), dtype=np.float32)
    wpad[:OUT_C] = w
    wpad[OUT_C:, 0] = 1.0  # unit-norm filler rows: no inf/nan anywhere

    in_maps = []
    for i in range(NCORES):
        sh = wpad[i * CSH : (i + 1) * CSH]
        wT = np.ascontiguousarray(sh.astype(ml_dtypes.bfloat16).T)
        loc = np.clip(tgt - i * CSH, 0, CSH - 1)
        wtgt = np.ascontiguousarray(sh[loc])
        in_maps.append({"wT": wT, "f": f32, "wtgt": wtgt})
    return in_maps, tgt


def kernel(features, targets, weights):
    global _GRAPH, LAST_EXEC_TIME_NS
    from concourse.bass_utils import run_bass_kernel_spmd

    if _GRAPH is None:
        _GRAPH = _build_graph()
    nc = _GRAPH

    in_maps, tgt = _prep_inputs(features, targets, weights)

    trace = bool(int(os.environ.get("BASS_KERNEL_TRACE", "0")))
    res = run_bass_kernel_spmd(nc, in_maps, core_ids=list(range(NCORES)), trace=trace)
    LAST_EXEC_TIME_NS = res.exec_time_ns

    outs = [res.results[i]["out"] for i in range(NCORES)]       # [CSH, B] f16
    full = np.concatenate(outs, axis=0)[:OUT_C]                 # [OUT_C, B]
    logits = np.ascontiguousarray(full.T, dtype=np.float32)     # [B, OUT_C]

    # apply device-computed margin deltas at the 1024 target positions
    deltas = np.stack(
        [res.results[i]["delta"].T.reshape(B) for i in range(NCORES)]
    )  # [NCORES, B]; delta[p, t] -> b = t*128 + p
    rows = np.arange(B)
    core_of = (tgt // CSH).astype(np.int64)
    logits[rows, tgt] += deltas[core_of, rows]
    return logits


# revision 3
# speedup vs baseline: 1.1970x; 1.1460x over previous
"""ArcFace logits on 8 Trainium2 NeuronCores — class-parallel (partial-FC) sharding.

Math: logits = SCALE * cos(arccos(clip(f_n @ w_n.T)) + MARGIN*onehot(targets))
Since cos(arccos(x)) == x, only the 1024 target entries need the margin
correction cos(t+m) = cos(m)*x - sin(m)*sqrt(1-x^2); everything else is just
the normalized matmul scaled by SCALE.

Device (SPMD, identical graph on all 8 cores, class-sharded):
  - f row-normalize (*SCALE folded in), cast bf16, PE-transpose -> fT
  - main matmul out[c,b] = wT.T @ fT in bf16 (fp32 PSUM), w-norm scaling fused
    into the PSUM->SBUF evacuation (per-partition tensor_scalar), fp16 out
  - w column norms: ACT Square -> k-presum (GPSIMD) -> one N=1 bf16
    ones-matmul per 128-class block -> DVE reciprocal + ACT Sqrt
  - margin deltas for all 1024 rows from gathered target weight rows
    (priority-demoted so the static scheduler slots it into mid-kernel slack)
DMA: all inputs + outputs on the two HWDGE rings (sync/scalar); weight chunks
prefetched two ahead; preamble loads (f, w0, w1) split across both rings.
Host: shard/transpose/concat + apply the device-computed deltas at the 1024
target positions (pure indexing; all arithmetic happens on device).
"""

import math
import os

import numpy as np

IN_F = 512
OUT_C = 100000
B = 1024
MARGIN = 0.5
SCALE = 20.0

NCORES = 8
CSH = 12800            # classes per core after padding
CPAD = CSH * NCORES    # 102400
P = 128
KT = IN_F // P         # 4 contraction subtiles
BT = B // P            # 8 batch tiles
NF = 512               # matmul moving free dim (one PSUM bank of fp32)
NB = B // NF           # 2
CW = 1280              # class chunk width streamed from DRAM
CBK = CW // P          # 10 class blocks per chunk
CHUNKS = CSH // CW     # 10
OG = 5                 # c-blocks batched per output DMA

_GRAPH = None
LAST_EXEC_TIME_NS = None


def _build_graph():
    from contextlib import ExitStack

    import concourse.bass as bass  # noqa: F401
    import concourse.tile as tile
    from concourse import bacc, mybir
    from concourse.masks import make_identity

    dt = mybir.dt
    AF = mybir.ActivationFunctionType
    ALU = mybir.AluOpType
    cosm = math.cos(MARGIN)
    sinm = math.sin(MARGIN)

    nc = bacc.Bacc()
    wT_e = nc.declare_dram_parameter("wT", [IN_F, CSH], dt.bfloat16, isOutput=False)
    f_e = nc.declare_dram_parameter("f", [B, IN_F], dt.bfloat16, isOutput=False)
    wtg_e = nc.declare_dram_parameter("wtgt", [B, IN_F], dt.bfloat16, isOutput=False)
    out_e = nc.declare_dram_parameter("out", [CSH, B], dt.float16, isOutput=True)
    dlt_e = nc.declare_dram_parameter("delta", [P, BT], dt.float32, isOutput=True)

    wT_v = wT_e[:].rearrange("(k p) c -> p k c", p=P)   # d = k*128 + p
    f_v = f_e[:].rearrange("(t p) d -> p t d", p=P)     # b = t*128 + p
    wtg_v = wtg_e[:].rearrange("(t p) d -> p t d", p=P)

    with ExitStack() as ctx:
        tc = ctx.enter_context(tile.TileContext(nc))
        cpool = ctx.enter_context(tc.tile_pool(name="cpool", bufs=1))
        fpool = ctx.enter_context(tc.tile_pool(name="fpool", bufs=1))
        wpool = ctx.enter_context(tc.tile_pool(name="wpool", bufs=3))
        sqpool = ctx.enter_context(tc.tile_pool(name="sqpool", bufs=2))
        opool = ctx.enter_context(tc.tile_pool(name="opool", bufs=3))
        smal = ctx.enter_context(tc.tile_pool(name="smal", bufs=2))
        pt_pool = ctx.enter_context(tc.tile_pool(name="pt", bufs=2, space="PSUM"))
        pn_pool = ctx.enter_context(tc.tile_pool(name="pn", bufs=1, space="PSUM"))
        po_pool = ctx.enter_context(tc.tile_pool(name="po", bufs=5, space="PSUM"))

        # ---------------- input DMAs first; f / w0 / w1 split across both
        # HWDGE rings so the preamble pipeline fills as fast as possible ----
        f_sb = fpool.tile([P, BT, IN_F], dt.bfloat16)
        nc.sync.dma_start(f_sb[:, 0 : BT // 2], f_v[:, 0 : BT // 2])
        nc.scalar.dma_start(f_sb[:, BT // 2 :], f_v[:, BT // 2 :])

        def emit_load(ci, split=False):
            w_sb = wpool.tile([P, KT, CW], dt.bfloat16, tag="wchunk", name="w_sb")
            src = wT_v[:, :, ci * CW : (ci + 1) * CW]
            if split:
                nc.sync.dma_start(w_sb[:, 0 : KT // 2], src[:, 0 : KT // 2])
                nc.scalar.dma_start(w_sb[:, KT // 2 :], src[:, KT // 2 :])
            else:
                eng = nc.sync if ci % 2 == 0 else nc.scalar
                eng.dma_start(w_sb[:], src)
            return w_sb

        w_sbs = {0: emit_load(0, split=True), 1: emit_load(1, split=True)}

        wt_sb = fpool.tile([P, BT, IN_F], dt.bfloat16, name="wt_sb")
        nc.sync.dma_start(wt_sb[:], wtg_v)

        ident = cpool.tile([P, P], dt.bfloat16)
        make_identity(nc, ident[:])
        ones_b = cpool.tile([P, 1], dt.bfloat16)
        nc.gpsimd.memset(ones_b[:], 1.0)

        # ---------------- f path: normalize, *SCALE, cast bf16, transpose ---
        nf2 = smal.tile([P, BT], dt.float32)
        for t in range(BT):
            sq = sqpool.tile([P, IN_F], dt.float32, tag="sqscratch")
            nc.scalar.activation(
                sq[:], f_sb[:, t], AF.Square, accum_out=nf2[:, t : t + 1]
            )
        rec_f = smal.tile([P, BT], dt.float32)
        nc.vector.reciprocal(rec_f[:], nf2[:])
        rnf20 = smal.tile([P, BT], dt.float32)
        # sqrt(SCALE^2 / nf2) = SCALE * rsqrt(nf2)
        nc.scalar.activation(rnf20[:], rec_f[:], AF.Sqrt, scale=SCALE * SCALE)

        f_ns = []
        for t in range(BT):
            f_n = fpool.tile([P, IN_F], dt.bfloat16, tag=f"fn{t}", name=f"fn{t}")
            nc.vector.tensor_scalar_mul(f_n[:], f_sb[:, t], rnf20[:, t : t + 1])
            f_ns.append(f_n)

        # ---------------- w-norm pipeline pieces --------------------------
        def emit_sqsum(ci, add_eng):
            """ACT square of chunk ci, then k-presum -> wsum bf16."""
            w_sb = w_sbs[ci]
            wsq = sqpool.tile([P, KT, CW], dt.bfloat16, tag="wsq", name="wsq")
            nc.scalar.activation(wsq[:], w_sb[:], AF.Square)
            wsum = sqpool.tile([P, CW], dt.bfloat16, tag="wsum", name="wsum")
            add_eng.tensor_add(wsum[:], wsq[:, 0], wsq[:, 1])
            add_eng.tensor_add(wsum[:], wsum[:], wsq[:, 2])
            add_eng.tensor_add(wsum[:], wsum[:], wsq[:, 3])
            return wsum

        # chunk0's norm inputs (emitted early, adds on DVE so they finish
        # before the fT copies saturate it)
        wsum0 = emit_sqsum(0, nc.vector)

        fT = fpool.tile([P, KT, B], dt.bfloat16)
        for t in range(BT):
            for k in range(KT):
                ps = pt_pool.tile([P, P], dt.bfloat16, tag="pst")
                nc.tensor.transpose(ps[:], f_ns[t][:, k * P : (k + 1) * P], ident[:])
                nc.vector.tensor_copy(fT[:, k, t * P : (t + 1) * P], ps[:])

        def emit_normmm(ci, wsum):
            """Per-class norm^2 via one N=1 ones-matmul per 128-class block."""
            nps = pn_pool.tile([P, CBK], dt.float32, tag="nps", name="nps")
            for cb in range(CBK):
                nc.tensor.matmul(
                    nps[:, cb : cb + 1],
                    lhsT=wsum[:, cb * P : (cb + 1) * P],
                    rhs=ones_b[:],
                    start=True,
                    stop=True,
                )
            recw = smal.tile([P, CBK], dt.float32, tag="recw", name="recw")
            nc.vector.reciprocal(recw[:], nps[:])
            rnw = smal.tile([P, CBK], dt.float32, tag="rnw", name="rnw")
            nc.scalar.activation(rnw[:], recw[:], AF.Sqrt, scale=1.0)
            return rnw

        def emit_margin():
            nt2 = smal.tile([P, BT], dt.float32)
            drot = smal.tile([P, BT], dt.float32)
            for t in range(BT):
                sq = sqpool.tile([P, IN_F], dt.float32, tag="sqscratch")
                nc.scalar.activation(
                    sq[:], wt_sb[:, t], AF.Square, accum_out=nt2[:, t : t + 1]
                )
                prod = sqpool.tile([P, IN_F], dt.float32, tag="prodscratch")
                nc.vector.tensor_mul(prod[:], f_sb[:, t], wt_sb[:, t])
                nc.vector.reduce_sum(
                    drot[:, t : t + 1], prod[:], axis=mybir.AxisListType.X
                )
            rec_t = smal.tile([P, BT], dt.float32)
            nc.vector.reciprocal(rec_t[:], nt2[:])
            rnt = smal.tile([P, BT], dt.float32)
            nc.scalar.activation(rnt[:], rec_t[:], AF.Sqrt, scale=1.0)
            u = smal.tile([P, BT], dt.float32)
            nc.vector.tensor_mul(u[:], drot[:], rnf20[:])
            nc.vector.tensor_mul(u[:], u[:], rnt[:])          # u = SCALE * cos_t
            t1 = smal.tile([P, BT], dt.float32)
            nc.vector.tensor_mul(t1[:], u[:], u[:])
            nc.vector.tensor_scalar(t1[:], t1[:], -1.0, SCALE * SCALE, ALU.mult, ALU.add)
            nc.vector.tensor_scalar_max(t1[:], t1[:], 0.0)    # max(S^2 - u^2, 0)
            s_t = smal.tile([P, BT], dt.float32)
            nc.scalar.activation(s_t[:], t1[:], AF.Sqrt, scale=1.0)  # SCALE*sin_t
            t2 = smal.tile([P, BT], dt.float32)
            nc.vector.tensor_scalar_mul(t2[:], s_t[:], -sinm)
            t3 = smal.tile([P, BT], dt.float32)
            nc.vector.tensor_scalar_mul(t3[:], u[:], cosm - 1.0)
            delta = smal.tile([P, BT], dt.float32)
            nc.vector.tensor_add(delta[:], t2[:], t3[:])
            nc.gpsimd.dma_start(dlt_e[:], delta[:])

        # ---------------- main class loop ---------------------------------
        # chunk0's norm runs in the preamble (PE waits on it once);
        # chunk i+1's square/presum is emitted at og1 of chunk i and its
        # norm-matmuls at the end of chunk i, so steady-state PE never waits.
        rnw = emit_normmm(0, wsum0)

        for ci in range(CHUNKS):
            w_sb = w_sbs.pop(ci)
            for og in range(CBK // OG):
                if og == 0 and ci + 2 < CHUNKS:
                    w_sbs[ci + 2] = emit_load(ci + 2)
                elif og == 1 and ci + 1 < CHUNKS:
                    next_wsum = emit_sqsum(ci + 1, nc.gpsimd)
                osb = opool.tile([P, OG, NB * NF], dt.float16, tag="osb")
                for cbi in range(OG):
                    cb = og * OG + cbi
                    psos = [
                        po_pool.tile([P, NF], dt.float32, tag="pso", name=f"pso{nb}")
                        for nb in range(NB)
                    ]
                    for k in range(KT):
                        for nb in range(NB):
                            nc.tensor.matmul(
                                psos[nb][:],
                                lhsT=w_sb[:, k, cb * P : (cb + 1) * P],
                                rhs=fT[:, k, nb * NF : (nb + 1) * NF],
                                start=(k == 0),
                                stop=(k == KT - 1),
                            )
                    for nb in range(NB):
                        if (cb + nb) % 2 == 0:
                            nc.scalar.activation(
                                osb[:, cbi, nb * NF : (nb + 1) * NF],
                                psos[nb][:], AF.Copy,
                                scale=rnw[:, cb : cb + 1],
                            )
                        else:
                            nc.vector.tensor_scalar_mul(
                                osb[:, cbi, nb * NF : (nb + 1) * NF],
                                psos[nb][:], rnw[:, cb : cb + 1],
                            )
                row0 = ci * CW + og * OG * P
                if ci == CHUNKS - 1 and og == 1:
                    # split the final burst across both rings to cut the tail
                    nc.sync.dma_start(
                        out_e[row0 : row0 + 2 * P, :].rearrange(
                            "(g p) b -> p g b", p=P
                        ),
                        osb[:, 0:2],
                    )
                    nc.scalar.dma_start(
                        out_e[row0 + 2 * P : row0 + OG * P, :].rearrange(
                            "(g p) b -> p g b", p=P
                        ),
                        osb[:, 2:OG],
                    )
                else:
                    dma_eng = nc.sync if og % 2 == 0 else nc.scalar
                    dma_eng.dma_start(
                        out_e[row0 : row0 + OG * P, :].rearrange(
                            "(g p) b -> p g b", p=P
                        ),
                        osb[:],
                    )
            if ci + 1 < CHUNKS:
                rnw = emit_normmm(ci + 1, next_wsum)
            if ci == 3:
                # demote so the static scheduler can't hoist these to the
                # front of the DVE/ACT streams (they'd head-of-line block
                # the f path while waiting on the wtgt DMA)
                with tc.high_priority(offset=-400):
                    emit_margin()

    nc.finalize()
    return nc


def _prep_inputs(features, targets, weights):
    import ml_dtypes

    f32 = np.asarray(features, dtype=np.float32)
    fbf = np.ascontiguousarray(f32.astype(ml_dtypes.bfloat16))
    tgt = np.asarray(targets).astype(np.int64)
    w = np.asarray(weights, dtype=np.float32)

    wpad = np.zeros((CPAD, IN_F), dtype=np.float32)
    wpad[:OUT_C] = w
    wpad[OUT_C:, 0] = 1.0  # unit-norm filler rows: no inf/nan anywhere

    in_maps = []
    for i in range(NCORES):
        sh = wpad[i * CSH : (i + 1) * CSH]
        wT = np.ascontiguousarray(sh.astype(ml_dtypes.bfloat16).T)
        loc = np.clip(tgt - i * CSH, 0, CSH - 1)
        wtgt = np.ascontiguousarray(sh[loc].astype(ml_dtypes.bfloat16))
        in_maps.append({"wT": wT, "f": fbf, "wtgt": wtgt})
    return in_maps, tgt


def kernel(features, targets, weights):
    global _GRAPH, LAST_EXEC_TIME_NS
    from concourse.bass_utils import run_bass_kernel_spmd

    if _GRAPH is None:
        _GRAPH = _build_graph()
    nc = _GRAPH

    in_maps, tgt = _prep_inputs(features, targets, weights)

    trace = bool(int(os.environ.get("BASS_KERNEL_TRACE", "0")))
    res = run_bass_kernel_spmd(nc, in_maps, core_ids=list(range(NCORES)), trace=trace)
    LAST_EXEC_TIME_NS = res.exec_time_ns

    outs = [res.results[i]["out"] for i in range(NCORES)]       # [CSH, B] f16
    full = np.concatenate(outs, axis=0)[:OUT_C]                 # [OUT_C, B]
    logits = np.ascontiguousarray(full.T, dtype=np.float32)     # [B, OUT_C]

    # apply device-computed margin deltas at the 1024 target positions
    deltas = np.stack(
        [res.results[i]["delta"].T.reshape(B) for i in range(NCORES)]
    )  # [NCORES, B]; delta[p, t] -> b = t*128 + p
    rows = np.arange(B)
    core_of = (tgt // CSH).astype(np.int64)
    logits[rows, tgt] += deltas[core_of, rows]
    return logits


# revision 9
# speedup vs baseline: 1.2862x; 1.0746x over previous
"""ArcFace logits on 8 Trainium2 NeuronCores — class-parallel (partial-FC) sharding.

Math: logits = SCALE * cos(arccos(clip(f_n @ w_n.T)) + MARGIN*onehot(targets))
Since cos(arccos(x)) == x, only the 1024 target entries need the margin
correction cos(t+m) = cos(m)*x - sin(m)*sqrt(1-x^2); everything else is just
the normalized matmul scaled by SCALE.

Device (SPMD, identical graph on all 8 cores, class-sharded):
  - f row-normalize (*SCALE folded in), cast bf16, PE-transpose -> fT
  - main matmul out[c,b] = wT.T @ fT in bf16 (fp32 PSUM), w-norm scaling fused
    into the PSUM->SBUF evacuation (per-partition tensor_scalar), fp16 out
  - w column norms: ACT Square -> k-presum (GPSIMD) -> one N=1 bf16
    ones-matmul per 128-class block -> DVE reciprocal + ACT Sqrt
  - margin deltas for all 1024 rows from gathered target weight rows
    (priority-demoted so the static scheduler slots it into mid-kernel slack)
DMA: all inputs + outputs on the two HWDGE rings (sync/scalar); weight chunks
prefetched two ahead; preamble loads (f, w0, w1) split across both rings.
Host: shard/transpose/concat + apply the device-computed deltas at the 1024
target positions (pure indexing; all arithmetic happens on device).
"""

import math
import os

import numpy as np

IN_F = 512
OUT_C = 100000
B = 1024
MARGIN = 0.5
SCALE = 20.0

NCORES = 8
CSH = 12800            # classes per core after padding
CPAD = CSH * NCORES    # 102400
P = 128
KT = IN_F // P         # 4 contraction subtiles
BT = B // P            # 8 batch tiles
NF = 512               # matmul moving free dim (one PSUM bank of fp32)
NB = B // NF           # 2
CW = 1280              # class chunk width streamed from DRAM
CBK = CW // P          # 10 class blocks per chunk
CHUNKS = CSH // CW     # 10
OG = 5                 # c-blocks batched per output DMA

_GRAPH = None
LAST_EXEC_TIME_NS = None


def _build_graph():
    from contextlib import ExitStack

    import concourse.bass as bass  # noqa: F401
    import concourse.tile as tile
    from concourse import bacc, mybir
    from concourse.masks import make_identity

    dt = mybir.dt
    AF = mybir.ActivationFunctionType
    ALU = mybir.AluOpType
    cosm = math.cos(MARGIN)
    sinm = math.sin(MARGIN)

    nc = bacc.Bacc()
    wT_e = nc.declare_dram_parameter("wT", [IN_F, CSH], dt.bfloat16, isOutput=False)
    f_e = nc.declare_dram_parameter("f", [B, IN_F], dt.bfloat16, isOutput=False)
    wtg_e = nc.declare_dram_parameter("wtgt", [B, IN_F], dt.bfloat16, isOutput=False)
    out_e = nc.declare_dram_parameter("out", [CSH, B], dt.float16, isOutput=True)
    dlt_e = nc.declare_dram_parameter("delta", [P, BT], dt.float32, isOutput=True)

    wT_v = wT_e[:].rearrange("(k p) c -> p k c", p=P)   # d = k*128 + p
    f_v = f_e[:].rearrange("(t p) d -> p t d", p=P)     # b = t*128 + p
    wtg_v = wtg_e[:].rearrange("(t p) d -> p t d", p=P)

    with ExitStack() as ctx:
        tc = ctx.enter_context(tile.TileContext(nc))
        cpool = ctx.enter_context(tc.tile_pool(name="cpool", bufs=1))
        fpool = ctx.enter_context(tc.tile_pool(name="fpool", bufs=1))
        wpool = ctx.enter_context(tc.tile_pool(name="wpool", bufs=3))
        sqpool = ctx.enter_context(tc.tile_pool(name="sqpool", bufs=2))
        opool = ctx.enter_context(tc.tile_pool(name="opool", bufs=3))
        smal = ctx.enter_context(tc.tile_pool(name="smal", bufs=2))
        pn_pool = ctx.enter_context(tc.tile_pool(name="pn", bufs=1, space="PSUM"))
        po_pool = ctx.enter_context(tc.tile_pool(name="po", bufs=6, space="PSUM"))

        # ---------------- input DMAs first; f / w0 / w1 split across both
        # HWDGE rings so the preamble pipeline fills as fast as possible ----
        f_sb = fpool.tile([P, BT, IN_F], dt.bfloat16)
        nc.sync.dma_start(f_sb[:, 0 : BT // 2], f_v[:, 0 : BT // 2])
        nc.scalar.dma_start(f_sb[:, BT // 2 :], f_v[:, BT // 2 :])

        def emit_load(ci, split=False):
            w_sb = wpool.tile([P, KT, CW], dt.bfloat16, tag="wchunk", name="w_sb")
            src = wT_v[:, :, ci * CW : (ci + 1) * CW]
            if split:
                nc.sync.dma_start(w_sb[:, 0 : KT // 2], src[:, 0 : KT // 2])
                nc.scalar.dma_start(w_sb[:, KT // 2 :], src[:, KT // 2 :])
            else:
                eng = nc.sync if ci % 2 == 0 else nc.scalar
                eng.dma_start(w_sb[:], src)
            return w_sb

        w_sbs = {0: emit_load(0, split=True), 1: emit_load(1, split=True)}

        wt_sb = fpool.tile([P, BT, IN_F], dt.bfloat16, name="wt_sb")
        nc.sync.dma_start(wt_sb[:], wtg_v)

        ident = cpool.tile([P, P], dt.bfloat16)
        make_identity(nc, ident[:])
        ones_b = cpool.tile([P, 1], dt.bfloat16)
        nc.gpsimd.memset(ones_b[:], 1.0)

        # ---------------- f path: normalize, *SCALE, cast bf16, transpose ---
        nf2 = smal.tile([P, BT], dt.float32)
        for t in range(BT):
            sq = sqpool.tile([P, IN_F], dt.float32, tag="sqscratch")
            nc.scalar.activation(
                sq[:], f_sb[:, t], AF.Square, accum_out=nf2[:, t : t + 1]
            )
        rec_f = smal.tile([P, BT], dt.float32)
        nc.vector.reciprocal(rec_f[:], nf2[:])
        rnf20 = smal.tile([P, BT], dt.float32)
        # sqrt(SCALE^2 / nf2) = SCALE * rsqrt(nf2)
        nc.scalar.activation(rnf20[:], rec_f[:], AF.Sqrt, scale=SCALE * SCALE)

        f_ns = []
        for t in range(BT):
            f_n = fpool.tile([P, IN_F], dt.bfloat16, tag=f"fn{t}", name=f"fn{t}")
            nc.vector.tensor_scalar_mul(f_n[:], f_sb[:, t], rnf20[:, t : t + 1])
            f_ns.append(f_n)

        # ---------------- w-norm pipeline pieces --------------------------
        def emit_sqsum(ci):
            """ACT square of chunk ci, then DVE k-presum -> wsum bf16."""
            w_sb = w_sbs[ci]
            wsq = sqpool.tile([P, KT, CW], dt.bfloat16, tag="wsq", name="wsq")
            nc.scalar.activation(wsq[:], w_sb[:], AF.Square)
            wsum = sqpool.tile([P, CW], dt.bfloat16, tag="wsum", name="wsum")
            nc.vector.tensor_add(wsum[:], wsq[:, 0], wsq[:, 1])
            nc.vector.tensor_add(wsum[:], wsum[:], wsq[:, 2])
            nc.vector.tensor_add(wsum[:], wsum[:], wsq[:, 3])
            return wsum

        # fT transposes: PSUM staging shares the po pool (frees banks for
        # matmul ILP); evacuating copies alternate DVE/ACT
        fT = fpool.tile([P, KT, B], dt.bfloat16)
        for t in range(BT):
            for k in range(KT):
                ps = po_pool.tile([P, P], dt.bfloat16, tag="pso", name="pst")
                nc.tensor.transpose(ps[:], f_ns[t][:, k * P : (k + 1) * P], ident[:])
                if (t * KT + k) % 2 == 0:
                    nc.vector.tensor_copy(fT[:, k, t * P : (t + 1) * P], ps[:])
                else:
                    nc.scalar.activation(
                        fT[:, k, t * P : (t + 1) * P], ps[:], AF.Copy
                    )

        # chunk0's presum (DVE, after the copies so it can't block them)
        wsum0 = emit_sqsum(0)

        def emit_normmm(ci, wsum):
            """Per-class norm^2 via one N=1 ones-matmul per 128-class block."""
            nps = pn_pool.tile([P, CBK], dt.float32, tag="nps", name="nps")
            for cb in range(CBK):
                nc.tensor.matmul(
                    nps[:, cb : cb + 1],
                    lhsT=wsum[:, cb * P : (cb + 1) * P],
                    rhs=ones_b[:],
                    start=True,
                    stop=True,
                )
            recw = smal.tile([P, CBK], dt.float32, tag="recw", name="recw")
            nc.vector.reciprocal(recw[:], nps[:])
            rnw = smal.tile([P, CBK], dt.float32, tag="rnw", name="rnw")
            nc.scalar.activation(rnw[:], recw[:], AF.Sqrt, scale=1.0)
            return rnw

        nt2 = smal.tile([P, BT], dt.float32)
        drot = smal.tile([P, BT], dt.float32)

        def emit_margin_part(ts):
            for t in ts:
                sq = sqpool.tile([P, IN_F], dt.float32, tag="sqscratch")
                nc.scalar.activation(
                    sq[:], wt_sb[:, t], AF.Square, accum_out=nt2[:, t : t + 1]
                )
                prod = sqpool.tile([P, IN_F], dt.float32, tag="prodscratch")
                nc.vector.tensor_mul(prod[:], f_sb[:, t], wt_sb[:, t])
                nc.vector.reduce_sum(
                    drot[:, t : t + 1], prod[:], axis=mybir.AxisListType.X
                )

        def emit_margin_tail():
            rec_t = smal.tile([P, BT], dt.float32)
            nc.vector.reciprocal(rec_t[:], nt2[:])
            rnt = smal.tile([P, BT], dt.float32)
            nc.scalar.activation(rnt[:], rec_t[:], AF.Sqrt, scale=1.0)
            u = smal.tile([P, BT], dt.float32)
            nc.vector.tensor_mul(u[:], drot[:], rnf20[:])
            nc.vector.tensor_mul(u[:], u[:], rnt[:])          # u = SCALE * cos_t
            t1 = smal.tile([P, BT], dt.float32)
            nc.vector.tensor_mul(t1[:], u[:], u[:])
            nc.vector.tensor_scalar(t1[:], t1[:], -1.0, SCALE * SCALE, ALU.mult, ALU.add)
            nc.vector.tensor_scalar_max(t1[:], t1[:], 0.0)    # max(S^2 - u^2, 0)
            s_t = smal.tile([P, BT], dt.float32)
            nc.scalar.activation(s_t[:], t1[:], AF.Sqrt, scale=1.0)  # SCALE*sin_t
            t2 = smal.tile([P, BT], dt.float32)
            nc.vector.tensor_scalar_mul(t2[:], s_t[:], -sinm)
            t3 = smal.tile([P, BT], dt.float32)
            nc.vector.tensor_scalar_mul(t3[:], u[:], cosm - 1.0)
            delta = smal.tile([P, BT], dt.float32)
            nc.vector.tensor_add(delta[:], t2[:], t3[:])
            nc.gpsimd.dma_start(dlt_e[:], delta[:])

        # ---------------- main class loop ---------------------------------
        # chunk0's norm runs in the preamble (PE waits on it once);
        # chunk i+1's square/presum is emitted at og1 of chunk i and its
        # norm-matmuls at the end of chunk i, so steady-state PE never waits.
        rnw = emit_normmm(0, wsum0)

        for ci in range(CHUNKS):
            w_sb = w_sbs.pop(ci)
            for og in range(CBK // OG):
                if og == 0 and ci + 2 < CHUNKS:
                    w_sbs[ci + 2] = emit_load(ci + 2)
                elif og == 1 and ci + 1 < CHUNKS:
                    next_wsum = emit_sqsum(ci + 1)
                osb = opool.tile([P, OG, NB * NF], dt.float16, tag="osb")
                for cbi in range(OG):
                    cb = og * OG + cbi
                    psos = [
                        po_pool.tile([P, NF], dt.float32, tag="pso", name=f"pso{nb}")
                        for nb in range(NB)
                    ]
                    for k in range(KT):
                        for nb in range(NB):
                            nc.tensor.matmul(
                                psos[nb][:],
                                lhsT=w_sb[:, k, cb * P : (cb + 1) * P],
                                rhs=fT[:, k, nb * NF : (nb + 1) * NF],
                                start=(k == 0),
                                stop=(k == KT - 1),
                            )
                    for nb in range(NB):
                        if (cb + nb) % 2 == 0:
                            nc.scalar.activation(
                                osb[:, cbi, nb * NF : (nb + 1) * NF],
                                psos[nb][:], AF.Copy,
                                scale=rnw[:, cb : cb + 1],
                            )
                        else:
                            nc.vector.tensor_scalar_mul(
                                osb[:, cbi, nb * NF : (nb + 1) * NF],
                                psos[nb][:], rnw[:, cb : cb + 1],
                            )
                row0 = ci * CW + og * OG * P
                if ci == CHUNKS - 1 and og == 1:
                    # split the final burst across both rings to cut the tail
                    nc.sync.dma_start(
                        out_e[row0 : row0 + 2 * P, :].rearrange(
                            "(g p) b -> p g b", p=P
                        ),
                        osb[:, 0:2],
                    )
                    nc.scalar.dma_start(
                        out_e[row0 + 2 * P : row0 + OG * P, :].rearrange(
                            "(g p) b -> p g b", p=P
                        ),
                        osb[:, 2:OG],
                    )
                else:
                    dma_eng = nc.sync if og % 2 == 0 else nc.scalar
                    dma_eng.dma_start(
                        out_e[row0 : row0 + OG * P, :].rearrange(
                            "(g p) b -> p g b", p=P
                        ),
                        osb[:],
                    )
            if ci + 1 < CHUNKS:
                rnw = emit_normmm(ci + 1, next_wsum)
            # margin work split into small slots across chunks so its
            # ACT/DVE bursts never starve the evacuations; demoted so the
            # static scheduler can't hoist it to the stream heads (it would
            # head-of-line block the f path waiting on the wtgt DMA)
            if 2 <= ci <= 5:
                with tc.high_priority(offset=-400):
                    emit_margin_part(range((ci - 2) * 2, (ci - 2) * 2 + 2))
            elif ci == 6:
                with tc.high_priority(offset=-400):
                    emit_margin_tail()

    nc.finalize()
    return nc


def _prep_inputs(features, targets, weights):
    import ml_dtypes

    f32 = np.asarray(features, dtype=np.float32)
    fbf = np.ascontiguousarray(f32.astype(ml_dtypes.bfloat16))
    tgt = np.asarray(targets).astype(np.int64)
    w = np.asarray(weights, dtype=np.float32)

    wpad = np.zeros((CPAD, IN_F), dtype=np.float32)
    wpad[:OUT_C] = w
    wpad[OUT_C:, 0] = 1.0  # unit-norm filler rows: no inf/nan anywhere

    in_maps = []
    for i in range(NCORES):
        sh = wpad[i * CSH : (i + 1) * CSH]
        wT = np.ascontiguousarray(sh.astype(ml_dtypes.bfloat16).T)
        loc = np.clip(tgt - i * CSH, 0, CSH - 1)
        wtgt = np.ascontiguousarray(sh[loc].astype(ml_dtypes.bfloat16))
        in_maps.append({"wT": wT, "f": fbf, "wtgt": wtgt})
    return in_maps, tgt


def kernel(features, targets, weights):
    global _GRAPH, LAST_EXEC_TIME_NS
    from concourse.bass_utils import run_bass_kernel_spmd

    if _GRAPH is None:
        _GRAPH = _build_graph()
    nc = _GRAPH

    in_maps, tgt = _prep_inputs(features, targets, weights)

    trace = bool(int(os.environ.get("BASS_KERNEL_TRACE", "0")))
    res = run_bass_kernel_spmd(nc, in_maps, core_ids=list(range(NCORES)), trace=trace)
    LAST_EXEC_TIME_NS = res.exec_time_ns

    outs = [res.results[i]["out"] for i in range(NCORES)]       # [CSH, B] f16
    full = np.concatenate(outs, axis=0)[:OUT_C]                 # [OUT_C, B]
    logits = np.ascontiguousarray(full.T, dtype=np.float32)     # [B, OUT_C]

    # apply device-computed margin deltas at the 1024 target positions
    deltas = np.stack(
        [res.results[i]["delta"].T.reshape(B) for i in range(NCORES)]
    )  # [NCORES, B]; delta[p, t] -> b = t*128 + p
    rows = np.arange(B)
    core_of = (tgt // CSH).astype(np.int64)
    logits[rows, tgt] += deltas[core_of, rows]
    return logits
